# revision 6
# baseline (speedup 1.0000x reference)
"""AttentionDownSample Trainium2 kernel v3 (8 NeuronCores, data-parallel over batch).

vs v2:
  - Software-pipelined emission: stage A(ch) = projections + softmax +
    transposes (+staging), stage B(ch) = broadcast + ui evacuation + weighted
    sum + output.  Emitted A(0), A(1), B(0), A(2), B(1), ... so chunk N+1's
    projection matmuls are ahead of chunk N's broadcast in the Tensor queue
    (kills head-of-line serialization seen in v2).
  - Broadcast matmuls write contiguous PSUM (strided dest halved PE rate).
  - ui evacuation split: 2 copies on scalar + 2 on gpsimd.
  - softmax normalize via one TT-divide (drops reciprocal+mult).
"""

import os
import sys

sys.path.insert(0, "/opt/trn_rl_repo")

import numpy as np

B, C, H, W = 8, 128, 256, 256
CR = 32
NH, NW = H // 2, W // 2
N_CORES = 8
R = 4                      # output rows per chunk
N_CHUNKS = NH // R         # 32
SLAB_ROWS_LIST = [8, 24, 32, 32, 32, 32, 32, 32, 32]  # tapered head
N_SLABS = len(SLAB_ROWS_LIST)
SLAB_BASE = [sum(SLAB_ROWS_LIST[:i]) for i in range(N_SLABS)]
CHUNKS_PER_SLAB_LIST = [r // (2 * R) for r in SLAB_ROWS_LIST]
CHUNK_SLAB = []  # chunk -> (slab, local row base)
for _s, _n in enumerate(CHUNKS_PER_SLAB_LIST):
    for _c in range(_n):
        CHUNK_SLAB.append((_s, _c * 2 * R))


def build_bass():
    import concourse.bass as bass
    import concourse.mybir as mybir
    from concourse import bacc, tile

    f32 = mybir.dt.float32
    bf16 = mybir.dt.bfloat16
    nc = bacc.Bacc()

    fm = nc.declare_dram_parameter("fm", [C, H, W], f32, isOutput=False)
    wqk = nc.declare_dram_parameter("wqk", [C, 2 * CR], f32, isOutput=False)
    ident = nc.declare_dram_parameter("ident", [128, 128], f32, isOutput=False)
    sel = nc.declare_dram_parameter("sel", [4, 4, 128], f32, isOutput=False)
    out = nc.declare_dram_parameter("out", [C, NH, NW], bf16, isOutput=True)

    with tile.TileContext(nc) as tc:
        with (
            tc.tile_pool(name="const", bufs=1) as cpool,
            tc.tile_pool(name="slabs", bufs=1) as spool,
            tc.tile_pool(name="work", bufs=3) as wpool,
            tc.tile_pool(name="psA", bufs=2, space="PSUM") as ppoolA,
            tc.tile_pool(name="psB", bufs=1, space="PSUM") as ppoolB,
        ):
            wqk_s = cpool.tile([C, 2 * CR], bf16)
            nc.gpsimd.dma_start(wqk_s[:], wqk[:])
            ident_s = cpool.tile([128, 128], bf16)
            nc.gpsimd.dma_start(ident_s[:], ident[:])
            sel_s = cpool.tile([4, 4, 128], bf16)
            nc.gpsimd.dma_start(sel_s[:], sel[:])

            # even/odd-strided loads: 1KB descriptors stream at ~325GB/s,
            # while contiguous 32KB-per-partition descriptors run ~50% idle.
            slabs = []
            for s in range(N_SLABS):
                rows = SLAB_ROWS_LIST[s]
                sl = spool.tile([C, rows, W], bf16, tag=f"slab{s}")
                base = SLAB_BASE[s]
                nc.gpsimd.dma_start(
                    sl[:, 0:rows:2, :],
                    fm[:, base : base + rows : 2, :],
                )
                nc.gpsimd.dma_start(
                    sl[:, 1:rows:2, :],
                    fm[:, base + 1 : base + rows : 2, :],
                )
                slabs.append(sl)

            state = {}  # per-chunk tiles passed from stage A to stage B

            def stage_a(ch):
                s, lr = CHUNK_SLAB[ch]
                sl = slabs[s]

                psum_k = ppoolA.tile([128, R, 4, CR], f32, tag="pk")
                psum_q = ppoolB.tile([128, R, CR], f32, tag="pq")
                for r in range(R):
                    for t in range(4):
                        dy, dx = t // 2, t % 2
                        xsl = sl[:, lr + 2 * r + dy, dx::2]
                        nc.tensor.matmul(
                            psum_k[:, r, t, :], xsl, wqk_s[:, 0:CR],
                            start=True, stop=True,
                        )
                        nc.tensor.matmul(
                            psum_q[:, r, :], xsl, wqk_s[:, CR : 2 * CR],
                            start=(t == 0), stop=(t == 3),
                        )

                qs = wpool.tile([128, R, CR], f32, tag="qs")
                nc.scalar.copy(qs[:], psum_q[:])
                prod = wpool.tile([128, R, 4, CR], f32, tag="prod")
                _q = qs[:]
                qs_b = bass.AP(
                    _q.tensor, _q.offset, _q.ap[:2] + [[0, 4]] + _q.ap[2:]
                )
                nc.vector.tensor_tensor(
                    prod[:], psum_k[:], qs_b, mybir.AluOpType.mult
                )
                logit = wpool.tile([128, R, 4], f32, tag="logit")
                nc.vector.tensor_reduce(
                    logit[:], prod[:], mybir.AxisListType.X, mybir.AluOpType.add
                )
                el = wpool.tile([128, R, 4], f32, tag="el")
                nc.scalar.activation(
                    el[:], logit[:], mybir.ActivationFunctionType.Exp
                )
                zsum = wpool.tile([128, R], f32, tag="zsum")
                nc.vector.tensor_reduce(
                    zsum[:], el[:], mybir.AxisListType.X, mybir.AluOpType.add
                )
                rz = wpool.tile([128, R], f32, tag="rz")
                nc.vector.reciprocal(rz[:], zsum[:])
                wgt = wpool.tile([128, R, 4], bf16, tag="wgt")
                _rz = rz[:]
                rz_b = bass.AP(_rz.tensor, _rz.offset, _rz.ap + [[0, 4]])
                nc.vector.tensor_tensor(
                    wgt[:], el[:], rz_b, mybir.AluOpType.mult
                )

                psum_wt = ppoolB.tile([4, R, 128], bf16, tag="pwt")
                for r in range(R):
                    nc.tensor.transpose(
                        psum_wt[:, r, :], wgt[:, r, :], ident_s[:]
                    )
                wts = wpool.tile([4, R, 128], bf16, tag="wts")
                nc.scalar.copy(wts[:], psum_wt[:])
                state[ch] = wts

            def stage_b(ch):
                s, lr = CHUNK_SLAB[ch]
                sl = slabs[s]
                wts = state.pop(ch)

                if ch % 4 == 0:
                    state["acc"] = wpool.tile(
                        [128, 4, R, NW], bf16, tag="acc", name="acc", bufs=2
                    )
                acc = state["acc"]

                # broadcast: natural layout per half-chunk, contiguous dest
                ui = wpool.tile([128, 2, R, W], bf16, tag="ui")
                pus = []
                for h in range(2):
                    pu = ppoolA.tile([128, 4, 2, 128], f32, tag="pu")
                    wts_h = wts[:, 2 * h : 2 * h + 2, :].rearrange(
                        "k r m -> k (r m)"
                    )  # [4, 256]
                    for t in range(4):
                        nc.tensor.matmul(
                            pu[:, t], sel_s[:, t, :], wts_h,
                            start=True, stop=True,
                        )
                    pus.append(pu)
                # evacuate + interleave, dy-major so ve (dy=0 consumer) can
                # start after two copies instead of three
                for dy in (0, 1):
                    for h in range(2):
                        pu = pus[h]
                        src = pu[:, 2 * dy : 2 * dy + 2, :, :]
                        d = ui[:, dy, 2 * h : 2 * h + 2, :]
                        dst = bass.AP(
                            d.tensor, d.offset,
                            [d.ap[0], [1, 2], [W, 2], [2, 128]],
                        )
                        nc.scalar.copy(dst, src)

                fe = sl[:, lr : lr + 2 * R : 2, :]
                fo = sl[:, lr + 1 : lr + 2 * R : 2, :]
                ve = wpool.tile([128, R, W], bf16, tag="ve")
                vo = wpool.tile([128, R, W], bf16, tag="vo")
                nc.vector.tensor_tensor(ve[:], fe, ui[:, 0], mybir.AluOpType.mult)
                nc.vector.tensor_tensor(vo[:], fo, ui[:, 1], mybir.AluOpType.mult)
                vs = wpool.tile([128, R, W], bf16, tag="vs")
                nc.vector.tensor_tensor(vs[:], ve[:], vo[:], mybir.AluOpType.add)
                nc.gpsimd.tensor_tensor(
                    acc[:, ch % 4], vs[:, :, 0::2], vs[:, :, 1::2],
                    mybir.AluOpType.add,
                )
                if ch % 4 == 3:
                    nc.sync.dma_start(
                        out[:, (ch - 3) * R : (ch + 1) * R, :],
                        acc[:].rearrange("p a r w -> p (a r) w"),
                    )

            # slab-batched emission: emit all A-stages of slab s, then all
            # B-stages of slab s-1.  B-work of a completed slab is never
            # queued behind A-work gated on a not-yet-arrived slab.
            stage_a(0)
            stage_a(1)
            for ch in range(2, N_CHUNKS):
                stage_a(ch)
                stage_b(ch - 2)
            stage_b(N_CHUNKS - 2)
            stage_b(N_CHUNKS - 1)

    nc.compile()
    return nc


_NC_CACHE = {}


def _get_nc():
    if "nc" not in _NC_CACHE:
        _NC_CACHE["nc"] = build_bass()
    return _NC_CACHE["nc"]


def _make_in_maps(fm, Wq, Wk):
    wq_eff = (Wq.astype(np.float64) * (CR ** -0.5) / 4.0).astype(np.float32)
    wqk = np.concatenate([Wk.astype(np.float32), wq_eff], axis=1)
    wqk = np.ascontiguousarray(wqk)
    ident = np.eye(128, dtype=np.float32)
    sel = np.zeros((4, 4, 128), dtype=np.float32)
    for t in range(4):
        sel[t, t, :] = 1.0
    return [
        {
            "fm": np.ascontiguousarray(fm[i]),
            "wqk": wqk,
            "ident": ident,
            "sel": sel,
        }
        for i in range(fm.shape[0])
    ]


def kernel(fm, Wq, Wk):
    from concourse.bass_utils import run_bass_kernel_spmd

    fm = np.asarray(fm, dtype=np.float32)
    Wq = np.asarray(Wq, dtype=np.float32)
    Wk = np.asarray(Wk, dtype=np.float32)

    nc = _get_nc()
    in_maps = _make_in_maps(fm, Wq, Wk)
    res = run_bass_kernel_spmd(nc, in_maps, core_ids=list(range(N_CORES)))
    outs = [np.asarray(res.results[i]["out"]).astype(np.float32) for i in range(N_CORES)]
    return np.stack(outs, axis=0)
